# revision 28
# baseline (speedup 1.0000x reference)
"""EngagementBiasedMHA on 8 Trainium2 NeuronCores.

Sharding: 4 batches x 2 head-groups (8 heads each).  Each core computes, for
its (batch, head-group):
  - K^T projection in [feat, token] layout and V projection in [token, feat]
    layout; V is stored per key-tile as [ones(64) | V_h] so the PV
    matmul also produces the softmax denominator on partitions 0:64
  - per 512-query chunk: attention in transposed layout: S^T = K @ Q^T with
    keys on partitions, so the per-key engagement bias/mask folds into the
    Exp activation as a per-partition bias, and exp(S^T) is already the
    correct (lhs-contraction) layout for the PV matmul
  - O^T = Vhat^T @ P^T accumulated over key tiles (rows 0:64 = replicated
    softmax denominator, rows 64:128 = head output)
  - row-parallel partial output projection y_partial = O_hg @ out_w.T[hg]
Matmul operands are bf16 (4x PE throughput vs fp32); accumulation stays fp32.

Schedule: the kernel is ACT(exp)-bound in steady state (256 exps of
[128,1024] at ~1.1us each).  All projection work except a minimal prologue
(K feats m=4 chunk 0 + Q feats m=0 chunk 0) is folded into the attention
loop as deadline-placed filler matmuls so the exp stream starts ~6us in and
the PE backfills projections in the per-period slack.  K/Q bias-adds run on
DVE (not ACT) so ACT does exps only.  Next-block S^T pairs are emitted
before evac/boundary work to avoid hp-boundary exp bubbles.

Host side: transpose/slice inputs per core, then sum the two partial outputs
per batch (row-parallel unshard).
"""

import sys

if "/opt/trn_rl_repo" not in sys.path:
    sys.path.insert(0, "/opt/trn_rl_repo")

import numpy as np
from concourse import bacc, tile
import concourse.mybir as mybir
from concourse.bass_utils import run_bass_kernel_spmd

F32 = mybir.dt.float32
BF16 = mybir.dt.bfloat16
NP_BF16 = mybir.dt.np(BF16)
AF = mybir.ActivationFunctionType

B, T, D, H = 4, 2048, 1024, 16
HD = 64
HG = 8           # heads per core
NKT = T // 128   # 16 key/token tiles
NQC = T // 512   # 4 query chunks
NDT = D // 128   # 8 d_in tiles
VROW = HG * 128  # 1024 Vhat columns per key tile: per head [ones(64) | V(64)]

_cache = {}

# Results of the most recent run (for the test harness to read exec times).
last_results = None


def _build_program():
    nc = bacc.Bacc("TRN2", target_bir_lowering=False, debug=False, num_devices=8)
    xt_d = nc.declare_dram_parameter("xt", [D, T], BF16, isOutput=False)
    # wqk: row block m*128+p holds, at col d*128+f, weight qkv_w.T[d*128+p, feat(m)+f]
    wqk_d = nc.declare_dram_parameter("wqk", [1024, 1024], BF16, isOutput=False)
    wv_d = nc.declare_dram_parameter("wv", [D, 512], BF16, isOutput=False)
    # small1 = [bqk(8) | eng(16) | maskf(16)] merged to one DMA
    small1_d = nc.declare_dram_parameter("small1", [128, 40], F32, isOutput=False)
    bv_d = nc.declare_dram_parameter("bv", [128, 512], F32, isOutput=False)
    wo_d = nc.declare_dram_parameter("wo", [512, 1024], BF16, isOutput=False)
    bo_d = nc.declare_dram_parameter("bo", [128, 1024], F32, isOutput=False)
    y_d = nc.declare_dram_parameter("y", [T, D], F32, isOutput=True)

    with tile.TileContext(nc) as tc:
        with (
            tc.tile_pool(name="persist", bufs=1) as persist,
            tc.tile_pool(name="wvpool", bufs=1) as wvpool,
            tc.tile_pool(name="wopool", bufs=1) as wopool,
            tc.tile_pool(name="small", bufs=1) as small,
            tc.tile_pool(name="ptpool", bufs=12) as ptpool,
            tc.tile_pool(name="otpool", bufs=9) as otpool,
            tc.tile_pool(name="evacpool", bufs=3) as evacpool,
            tc.tile_pool(name="recpool", bufs=3) as recpool,
            tc.tile_pool(name="psmix", bufs=2, space="PSUM") as psmix,
            tc.tile_pool(name="psops", bufs=2, space="PSUM") as psops,
            tc.tile_pool(name="psST", bufs=2, space="PSUM") as psST,
        ):
            # ---- resident activations / weights (bf16) ----
            XT = persist.tile([128, NDT * T], BF16, name="XT")
            WQK = persist.tile([128, 8 * 1024], BF16, name="WQK")
            WV = wvpool.tile([128, NDT * 512], BF16, name="WV")
            WO = wopool.tile([128, 4 * 1024], BF16, name="WO")
            SM1 = small.tile([128, 40], F32, name="SM1")
            BV = small.tile([128, 512], F32, name="BV")
            BO = small.tile([128, 1024], F32, name="BO")
            QTKT = persist.tile([128, 8 * T], BF16, name="QTKT")
            VHAT = persist.tile([128, NKT * VROW], BF16, name="VHAT")

            def dma_wqk(eng, m, splits=1):
                # split across partition ranges -> parallel DMA queues
                step = 128 // splits
                for s in range(splits):
                    eng.dma_start(
                        WQK[s * step:(s + 1) * step, m * 1024:(m + 1) * 1024],
                        wqk_d[m * 128 + s * step: m * 128 + (s + 1) * step, :])

            def dma_xt_chunk(eng, c):
                for d in range(NDT):
                    eng.dma_start(
                        XT[:, d * T + c * 512: d * T + (c + 1) * 512],
                        xt_d[d * 128:(d + 1) * 128, c * 512:(c + 1) * 512])

            # Each dma_start costs ~0.7us of serialized dispatch on the
            # issuing engine's sequencer, so the input DMAs are spread over
            # three dispatch lanes (sync / gpsimd / vector), deadline-ordered
            # within each lane.
            # sync lane: all XT chunks in deadline order (they feed the
            # S-pair stream directly), then K m5 / Q m1
            nc.sync.dma_start(SM1[:], small1_d[:])
            dma_wqk(nc.sync, 4, splits=2)
            dma_wqk(nc.sync, 0, splits=2)
            dma_xt_chunk(nc.sync, 0)
            dma_xt_chunk(nc.sync, 1)
            dma_xt_chunk(nc.sync, 2)
            dma_xt_chunk(nc.sync, 3)
            dma_wqk(nc.sync, 5)
            dma_wqk(nc.sync, 1)
            # BK prep first in the vector/scalar queues (needs only SM1):
            # BK = ln(max(eng, 1e-6)) - 1e9 * mask
            BQK = SM1[:, 0:8]
            ENG = SM1[:, 8:24]
            MSK = SM1[:, 24:40]
            BK = small.tile([128, NKT], F32, name="BK")
            nc.vector.tensor_scalar_max(BK[:], ENG, 1e-6)
            nc.scalar.activation(BK[:], BK[:], AF.Ln)
            MK9 = small.tile([128, NKT], F32, name="MK9")
            nc.vector.tensor_scalar_mul(MK9[:], MSK, -1e9)
            nc.vector.tensor_add(BK[:], BK[:], MK9[:])

            # The gpsimd DMA lane is held back behind the last xt chunk-0
            # tile so its descriptors don't contend with the prologue-
            # critical transfers on the shared DMA queues.  (Never gate the
            # scalar engine: its queue must stay clear for the exp stream.)
            GATE = small.tile([1, 8], F32, name="GATE")
            nc.gpsimd.tensor_scalar_add(GATE[0:1, 0:4], XT[0:1, 7 * T: 7 * T + 4], 0.0)

            # gpsimd lane: WV, then late weights
            for d in range(NDT):
                nc.gpsimd.dma_start(WV[:, d * 512:(d + 1) * 512],
                                    wv_d[d * 128:(d + 1) * 128, :])
            for s in range(2):
                nc.gpsimd.dma_start(BV[s * 64:(s + 1) * 64, :],
                                    bv_d[s * 64:(s + 1) * 64, :])
            dma_wqk(nc.gpsimd, 6)
            dma_wqk(nc.gpsimd, 7)
            dma_wqk(nc.gpsimd, 2)
            dma_wqk(nc.gpsimd, 3)
            for s in range(2):
                nc.gpsimd.dma_start(BO[s * 64:(s + 1) * 64, :],
                                    bo_d[s * 64:(s + 1) * 64, :])
            for f in range(4):
                nc.gpsimd.dma_start(WO[:, f * 1024:(f + 1) * 1024],
                                    wo_d[f * 128:(f + 1) * 128, :])
            # VHAT ones memsets on the vector engine
            for t in range(NKT):
                nc.vector.memset(VHAT[:, t * VROW:(t + 1) * VROW], 1.0)



            # ---- projection helpers (bias-add on DVE, not ACT) ----
            def proj_qk(m, c):
                # K feats (m=4..7) or Q feats (m=0..3) for token chunk c
                ps = psmix.tile([128, 512], F32, name="ps_qk", tag="mix")
                for d in range(NDT):
                    nc.tensor.matmul(
                        ps[:],
                        lhsT=WQK[:, m * 1024 + d * 128: m * 1024 + (d + 1) * 128],
                        rhs=XT[:, d * T + c * 512: d * T + c * 512 + 512],
                        start=(d == 0), stop=(d == NDT - 1),
                    )
                nc.vector.tensor_scalar_add(
                    QTKT[:, m * T + c * 512: m * T + c * 512 + 512],
                    ps[:], BQK[:, m:m + 1])

            def proj_v(t, pair):
                # V feats for heads [4*pair, 4*pair+4), token tile t (N=256)
                ps = psmix.tile([128, 256], F32, name="ps_v", tag="mix")
                for d in range(NDT):
                    nc.tensor.matmul(
                        ps[:],
                        lhsT=XT[:, d * T + t * 128: d * T + (t + 1) * 128],
                        rhs=WV[:, d * 512 + pair * 256: d * 512 + (pair + 1) * 256],
                        start=(d == 0), stop=(d == NDT - 1),
                    )
                vslice = VHAT[:, t * VROW + pair * 512: t * VROW + (pair + 1) * 512
                              ].rearrange("p (h c) -> p h c", c=128)[:, :, 64:128]
                nc.vector.tensor_add(
                    vslice,
                    ps[:].rearrange("p (h c) -> p h c", c=64),
                    BV[:, pair * 256:(pair + 1) * 256].rearrange(
                        "p (h c) -> p h c", c=64))

            def out_proj(qc2, otc2, grp):
                t4, c2 = grp // 2, grp % 2
                tt = qc2 * 4 + t4
                ps = psmix.tile([128, 512], F32, name="ps_y", tag="mix")
                for f in range(4):
                    nc.tensor.matmul(
                        ps[:],
                        lhsT=otc2[f][:, t4 * 128:(t4 + 1) * 128],
                        rhs=WO[:, f * 1024 + c2 * 512: f * 1024 + c2 * 512 + 512],
                        start=(f == 0), stop=(f == 3))
                yv = evacpool.tile([128, 512], F32, name="yv", tag="yv")
                nc.vector.tensor_add(yv[:], ps[:], BO[:, c2 * 512:(c2 + 1) * 512])
                nc.sync.dma_start(
                    y_d[tt * 128:(tt + 1) * 128, c2 * 512:(c2 + 1) * 512], yv[:])

            # ---- prologue: just enough for the exp stream to start ----
            proj_qk(4, 0)   # K feats for hp0, token chunk 0
            proj_qk(0, 0)   # Q feats for qt0, query chunk 0

            # ---- block order: interleave qc0/qc1 (then qc2/qc3) so the
            # projection fillers' deadlines spread over 128 periods instead
            # of crowding into the first 64 ----
            BLOCKS = [(0, 0), (0, 1), (1, 0), (1, 1), (0, 2), (0, 3), (1, 2), (1, 3),
                      (2, 0), (2, 1), (3, 0), (3, 1), (2, 2), (2, 3), (3, 2), (3, 3)]

            # ---- filler schedule: block index, kt -> list of thunks ----
            fillers = {}

            def add_filler(bi, kt, thunk):
                fillers.setdefault((bi, kt), []).append(thunk)

            # K feature tiles, chunk-granular just-in-time:
            #   m=4: prologue c0; c1/c2/c3 inside blk0 (first hp0 block)
            #   m=5: c0 at blk0 tail; c1/2/3 inside blk1 (first hp1 block)
            #   m=6: c0 at blk3 tail; c1/2/3 inside blk4 (first hp2 block)
            #   m=7: c0 at blk4 tail; c1/2/3 inside blk5 (first hp3 block)
            for c in range(1, 4):
                add_filler(0, 4 * c - 3, (lambda c=c: proj_qk(4, c)))
                add_filler(1, 4 * c - 3, (lambda c=c: proj_qk(5, c)))
                add_filler(4, 4 * c - 3, (lambda c=c: proj_qk(6, c)))
                add_filler(5, 4 * c - 3, (lambda c=c: proj_qk(7, c)))
            add_filler(0, 10, (lambda: proj_qk(5, 0)))
            add_filler(3, 10, (lambda: proj_qk(6, 0)))
            add_filler(4, 10, (lambda: proj_qk(7, 0)))
            # Q-tiles JIT: block bi needs Q(m=hp, c=qc); emit one block ahead
            for bi in range(1, 16):
                qc, hp = BLOCKS[bi]
                add_filler(bi - 1, 11, (lambda hp=hp, qc=qc: proj_qk(hp, qc)))
            # V projection: pair0 (heads 0..3) during blk0, pair1 during blk4
            for t in range(NKT):
                add_filler(0, t, (lambda t=t: proj_v(t, 0)))
                add_filler(4, t, (lambda t=t: proj_v(t, 1)))

            # deferred out-proj groups: qc ready after its last block's evac
            # (qc0 after blk5, qc1 after blk7, qc2 after blk13, qc3 at end).
            # Groups are spread across the NEXT block's kts as fillers so the
            # boundary never dumps a multi-us matmul burst in front of the
            # S-pairs (which would drain the 2-deep st buffer and stall ACT).
            outproj_spread = {
                6: [(0, 0), (0, 1)], 7: [(0, 2), (0, 3)], 8: [(0, 4), (0, 5)],
                9: [(0, 6), (0, 7)],
                10: [(1, 0), (1, 1)], 11: [(1, 2), (1, 3)], 12: [(1, 4), (1, 5)],
                13: [(1, 6), (1, 7), (2, 0), (2, 1)],
                14: [(2, 2), (2, 3), (2, 4), (2, 5), (2, 6), (2, 7)],
            }
            _slots = {2: (2, 8), 4: (2, 5, 8, 11), 6: (2, 4, 6, 8, 10, 12)}
            for bi, work in outproj_spread.items():
                for j, (oqc, g) in enumerate(work):
                    add_filler(bi + 1, _slots[len(work)][j],
                               (lambda oqc=oqc, g=g: out_proj(
                                   oqc, [otc_by_qc[oqc][f] for f in range(4)], g)))

            state = {}
            otc_by_qc = {}

            def emit_block_tail(bi):
                qc, hp = BLOCKS[bi]
                ops = state.pop(bi)["ops"]
                OTc = otpool.tile([128, 512], BF16, name="OTc", tag="otc")
                for sub in range(2):
                    rec = recpool.tile([64, 512], F32, name="rec", tag="rec")
                    nc.vector.reciprocal_approx_fast(rec[:], ops[sub][0:64, :])
                    nc.vector.tensor_mul(
                        OTc[sub * 64:sub * 64 + 64, :],
                        ops[sub][64:128, :], rec[:])
                otc_by_qc.setdefault(qc, {})[hp] = OTc

            def pv_pair(bi, kt, pt):
                qc, hp = BLOCKS[bi]
                ops = state[bi]["ops"]
                for sub in range(2):
                    h = 2 * hp + sub
                    nc.tensor.matmul(
                        ops[sub][:],
                        lhsT=VHAT[:, kt * VROW + h * 128: kt * VROW + (h + 1) * 128],
                        rhs=pt[:, sub * 512:(sub + 1) * 512],
                        start=(kt == 0), stop=(kt == NKT - 1))

            # ---- attention: flattened pipeline over BLOCKS x kt ----
            # Per index: S-pair, exp, prev-block tail (at kt==1), fillers,
            # then the PV pair LAGGED by one iteration so it never blocks
            # the in-order PE queue waiting on the exp or the evac.
            pts = {}
            for i in range(len(BLOCKS) * NKT):
                bi, kt = i // NKT, i % NKT
                qc, hp = BLOCKS[bi]
                qt = hp
                ktf = 4 + hp
                if kt == 0:
                    op0 = psops.tile([128, 512], F32, name="op0", tag="ops")
                    op1 = psops.tile([128, 512], F32, name="op1", tag="ops")
                    state[bi] = {"ops": (op0, op1)}

                # S^T pair (the two K=64 matmuls run concurrently via
                # base_partition-derived PE row tiling)
                st = psST.tile([128, 1024], F32, name="st", tag="st")
                for sub in range(2):
                    lo = sub * 64
                    nc.tensor.matmul(
                        st[:, sub * 512:(sub + 1) * 512],
                        lhsT=QTKT[lo:lo + 64, ktf * T + kt * 128: ktf * T + (kt + 1) * 128],
                        rhs=QTKT[lo:lo + 64, qt * T + qc * 512: qt * T + qc * 512 + 512],
                        start=True, stop=True)
                pt = ptpool.tile([128, 1024], BF16, name="pt", tag="pt")
                nc.scalar.activation(
                    pt[:], st[:], AF.Exp,
                    bias=BK[:, kt:kt + 1], scale=0.125)
                pts[(bi, kt)] = pt

                if kt == 2 and bi > 0:
                    emit_block_tail(bi - 1)
                for th in fillers.get((bi, kt), ()):
                    th()

                # PV lagged TWO iterations: its exp input is then two full
                # periods old, so the in-order PE queue never waits on ACT.
                j = i - 2
                if j >= 0:
                    bj, ktj = divmod(j, NKT)
                    pv_pair(bj, ktj, pts.pop((bj, ktj)))

            pv_pair(15, 14, pts.pop((15, 14)))
            pv_pair(15, 15, pts.pop((15, 15)))
            emit_block_tail(15)
            # qc3's out-projs drain at the end
            otc3 = [otc_by_qc[3][f] for f in range(4)]
            for grp in range(8):
                out_proj(3, otc3, grp)
    nc.compile()
    return nc


def get_program():
    if "nc" not in _cache:
        _cache["nc"] = _build_program()
    return _cache["nc"]


def shard_inputs(x, engagement, mask, qkv_w, qkv_b, out_w, out_b):
    """Build the per-core input maps (host-side layout prep only)."""
    x = np.asarray(x, dtype=np.float32)
    engagement = np.asarray(engagement, dtype=np.float32)
    maskf = np.asarray(mask).astype(np.float32)
    qkv_w = np.asarray(qkv_w, dtype=np.float32)
    qkv_b = np.asarray(qkv_b, dtype=np.float32)
    out_w = np.asarray(out_w, dtype=np.float32)
    out_b = np.asarray(out_b, dtype=np.float32)

    qkvT = qkv_w.T  # [D, 3D]
    outT = out_w.T  # [D, D]
    in_maps = []
    for cix in range(8):
        b, hg = cix // 2, cix % 2
        qcols = qkvT[:, hg * 512:(hg + 1) * 512]
        kcols = qkvT[:, 1024 + hg * 512: 1024 + (hg + 1) * 512]
        sel = np.concatenate([qcols, kcols], axis=1)  # [1024 din, 1024 feats]
        # [d, p, m, f] -> [m, p, d, f] -> [(m p), (d f)]
        wqk = sel.reshape(NDT, 128, 8, 128).transpose(2, 1, 0, 3).reshape(1024, 1024)
        bq = qkv_b[hg * 512:(hg + 1) * 512].reshape(4, 128).T
        bk = qkv_b[1024 + hg * 512: 1024 + (hg + 1) * 512].reshape(4, 128).T
        bo = np.broadcast_to(out_b, (128, 1024)) if hg == 0 else np.zeros((128, 1024), np.float32)
        small1 = np.concatenate(
            [bq, bk,
             engagement[b].reshape(NKT, 128).T,
             maskf[b].reshape(NKT, 128).T], axis=1)
        in_maps.append({
            "xt": np.ascontiguousarray(x[b].T).astype(NP_BF16),
            "wqk": np.ascontiguousarray(wqk).astype(NP_BF16),
            "wv": np.ascontiguousarray(
                qkvT[:, 2048 + hg * 512: 2048 + (hg + 1) * 512]).astype(NP_BF16),
            "small1": np.ascontiguousarray(small1),
            "bv": np.ascontiguousarray(
                np.broadcast_to(qkv_b[2048 + hg * 512: 2048 + (hg + 1) * 512], (128, 512))),
            "wo": np.ascontiguousarray(outT[hg * 512:(hg + 1) * 512, :]).astype(NP_BF16),
            "bo": np.ascontiguousarray(bo),
        })
    return in_maps


def kernel(x, engagement, mask, qkv_w, qkv_b, out_w, out_b):
    global last_results
    nc = get_program()
    in_maps = shard_inputs(x, engagement, mask, qkv_w, qkv_b, out_w, out_b)
    res = run_bass_kernel_spmd(nc, in_maps, list(range(8)))
    last_results = res
    out = np.empty((B, T, D), dtype=np.float32)
    for b in range(B):
        out[b] = res.results[2 * b]["y"] + res.results[2 * b + 1]["y"]
    return out


# revision 31
# speedup vs baseline: 1.1694x; 1.1694x over previous
"""EngagementBiasedMHA on 8 Trainium2 NeuronCores.

Sharding: 4 batches x 2 head-groups (8 heads each).  Each core computes, for
its (batch, head-group):
  - K^T projection in [feat, token] layout and V projection in [token, feat]
    layout; V is stored per key-tile as [ones(64) | V_h] so the PV
    matmul also produces the softmax denominator on partitions 0:64
  - per 512-query chunk: attention in transposed layout: S^T = K @ Q^T with
    keys on partitions, so the per-key engagement bias/mask folds into the
    Exp activation as a per-partition bias, and exp(S^T) is already the
    correct (lhs-contraction) layout for the PV matmul
  - O^T = Vhat^T @ P^T accumulated over key tiles (rows 0:64 = replicated
    softmax denominator, rows 64:128 = head output)
  - row-parallel partial output projection y_partial = O_hg @ out_w.T[hg]
Matmul operands are bf16 (4x PE throughput vs fp32); accumulation stays fp32.

Schedule: the kernel is ACT(exp)-bound in steady state (256 exps of
[128,1024] at ~1.1us each).  All projection work except a minimal prologue
(K feats m=4 chunk 0 + Q feats m=0 chunk 0) is folded into the attention
loop as deadline-placed filler matmuls so the exp stream starts ~6us in and
the PE backfills projections in the per-period slack.  K/Q bias-adds run on
DVE (not ACT) so ACT does exps only.  Next-block S^T pairs are emitted
before evac/boundary work to avoid hp-boundary exp bubbles.

Host side: transpose/slice inputs per core, then sum the two partial outputs
per batch (row-parallel unshard).
"""

import sys

if "/opt/trn_rl_repo" not in sys.path:
    sys.path.insert(0, "/opt/trn_rl_repo")

import numpy as np
from concourse import bacc, tile
import concourse.mybir as mybir
from concourse.bass_utils import run_bass_kernel_spmd

F32 = mybir.dt.float32
BF16 = mybir.dt.bfloat16
NP_BF16 = mybir.dt.np(BF16)
AF = mybir.ActivationFunctionType

B, T, D, H = 4, 2048, 1024, 16
HD = 64
HG = 8           # heads per core
NKT = T // 128   # 16 key/token tiles
NQC = T // 512   # 4 query chunks
NDT = D // 128   # 8 d_in tiles
VROW = HG * 128  # 1024 Vhat columns per key tile: per head [ones(64) | V(64)]

_cache = {}

# Results of the most recent run (for the test harness to read exec times).
last_results = None


def _build_program():
    nc = bacc.Bacc("TRN2", target_bir_lowering=False, debug=False, num_devices=8)
    xt_d = nc.declare_dram_parameter("xt", [D, T], BF16, isOutput=False)
    # wqk: row block m*128+p holds, at col d*128+f, weight qkv_w.T[d*128+p, feat(m)+f]
    wqk_d = nc.declare_dram_parameter("wqk", [1024, 1024], BF16, isOutput=False)
    wv_d = nc.declare_dram_parameter("wv", [D, 512], BF16, isOutput=False)
    # small1 = [bqk(8) | eng(16) | maskf(16)] merged to one DMA
    small1_d = nc.declare_dram_parameter("small1", [128, 40], F32, isOutput=False)
    bv_d = nc.declare_dram_parameter("bv", [128, 512], F32, isOutput=False)
    wo_d = nc.declare_dram_parameter("wo", [512, 1024], BF16, isOutput=False)
    bo_d = nc.declare_dram_parameter("bo", [128, 1024], F32, isOutput=False)
    y_d = nc.declare_dram_parameter("y", [T, D], F32, isOutput=True)

    with tile.TileContext(nc) as tc:
        with (
            tc.tile_pool(name="persist", bufs=1) as persist,
            tc.tile_pool(name="wvpool", bufs=1) as wvpool,
            tc.tile_pool(name="wopool", bufs=1) as wopool,
            tc.tile_pool(name="small", bufs=1) as small,
            tc.tile_pool(name="ptpool", bufs=12) as ptpool,
            tc.tile_pool(name="otpool", bufs=9) as otpool,
            tc.tile_pool(name="evacpool", bufs=3) as evacpool,
            tc.tile_pool(name="recpool", bufs=3) as recpool,
            tc.tile_pool(name="psmix", bufs=2, space="PSUM") as psmix,
            tc.tile_pool(name="psops", bufs=2, space="PSUM") as psops,
            tc.tile_pool(name="psST", bufs=2, space="PSUM") as psST,
        ):
            # ---- resident activations / weights (bf16) ----
            XT = persist.tile([128, NDT * T], BF16, name="XT")
            WQK = persist.tile([128, 8 * 1024], BF16, name="WQK")
            WV = wvpool.tile([128, NDT * 512], BF16, name="WV")
            WO = wopool.tile([128, 4 * 1024], BF16, name="WO")
            SM1 = small.tile([128, 40], F32, name="SM1")
            BV = small.tile([128, 512], F32, name="BV")
            BO = small.tile([128, 1024], F32, name="BO")
            QTKT = persist.tile([128, 8 * T], BF16, name="QTKT")
            VHAT = persist.tile([128, NKT * VROW], BF16, name="VHAT")

            def dma_wqk(eng, m, splits=1):
                # split across partition ranges -> parallel DMA queues
                step = 128 // splits
                for s in range(splits):
                    eng.dma_start(
                        WQK[s * step:(s + 1) * step, m * 1024:(m + 1) * 1024],
                        wqk_d[m * 128 + s * step: m * 128 + (s + 1) * step, :])

            def dma_xt_chunk(eng, c):
                for d in range(NDT):
                    eng.dma_start(
                        XT[:, d * T + c * 512: d * T + (c + 1) * 512],
                        xt_d[d * 128:(d + 1) * 128, c * 512:(c + 1) * 512])

            # Each dma_start costs ~0.7us of serialized dispatch on the
            # issuing engine's sequencer, so the input DMAs are spread over
            # three dispatch lanes (sync / gpsimd / vector), deadline-ordered
            # within each lane.
            # sync lane: the prologue-critical transfers first (K m4 weights,
            # then xt chunk 0 so the K d-loop can start streaming), then the
            # remaining XT chunks in deadline order, then K m5 / Q m1
            dma_wqk(nc.sync, 4, splits=2)
            dma_xt_chunk(nc.sync, 0)
            dma_wqk(nc.sync, 0, splits=2)
            nc.sync.dma_start(SM1[:], small1_d[:])
            dma_xt_chunk(nc.sync, 1)
            dma_xt_chunk(nc.sync, 2)
            dma_xt_chunk(nc.sync, 3)
            dma_wqk(nc.sync, 5)
            dma_wqk(nc.sync, 1)
            # BK prep first in the vector/scalar queues (needs only SM1):
            # BK = ln(max(eng, 1e-6)) - 1e9 * mask
            BQK = SM1[:, 0:8]
            ENG = SM1[:, 8:24]
            MSK = SM1[:, 24:40]
            BK = small.tile([128, NKT], F32, name="BK")
            nc.vector.tensor_scalar_max(BK[:], ENG, 1e-6)
            nc.scalar.activation(BK[:], BK[:], AF.Ln)
            MK9 = small.tile([128, NKT], F32, name="MK9")
            nc.vector.tensor_scalar_mul(MK9[:], MSK, -1e9)
            nc.vector.tensor_add(BK[:], BK[:], MK9[:])

            # The gpsimd DMA lane is held back behind the last xt chunk-0
            # tile so its descriptors don't contend with the prologue-
            # critical transfers on the shared DMA queues.  (Never gate the
            # scalar engine: its queue must stay clear for the exp stream.)
            GATE = small.tile([1, 8], F32, name="GATE")
            nc.gpsimd.tensor_scalar_add(GATE[0:1, 0:4], XT[0:1, 7 * T: 7 * T + 4], 0.0)

            # gpsimd lane: WV, then late weights
            for d in range(NDT):
                nc.gpsimd.dma_start(WV[:, d * 512:(d + 1) * 512],
                                    wv_d[d * 128:(d + 1) * 128, :])
            for s in range(2):
                nc.gpsimd.dma_start(BV[s * 64:(s + 1) * 64, :],
                                    bv_d[s * 64:(s + 1) * 64, :])
            dma_wqk(nc.gpsimd, 6)
            dma_wqk(nc.gpsimd, 7)
            dma_wqk(nc.gpsimd, 2)
            dma_wqk(nc.gpsimd, 3)
            for s in range(2):
                nc.gpsimd.dma_start(BO[s * 64:(s + 1) * 64, :],
                                    bo_d[s * 64:(s + 1) * 64, :])
            for f in range(4):
                nc.gpsimd.dma_start(WO[:, f * 1024:(f + 1) * 1024],
                                    wo_d[f * 128:(f + 1) * 128, :])
            # VHAT ones memsets on the vector engine
            for t in range(NKT):
                nc.vector.memset(VHAT[:, t * VROW:(t + 1) * VROW], 1.0)



            # ---- projection helpers (bias-add on DVE, not ACT) ----
            def proj_qk(m, c):
                # K feats (m=4..7) or Q feats (m=0..3) for token chunk c
                ps = psmix.tile([128, 512], F32, name="ps_qk", tag="mix")
                for d in range(NDT):
                    nc.tensor.matmul(
                        ps[:],
                        lhsT=WQK[:, m * 1024 + d * 128: m * 1024 + (d + 1) * 128],
                        rhs=XT[:, d * T + c * 512: d * T + c * 512 + 512],
                        start=(d == 0), stop=(d == NDT - 1),
                    )
                nc.vector.tensor_scalar_add(
                    QTKT[:, m * T + c * 512: m * T + c * 512 + 512],
                    ps[:], BQK[:, m:m + 1])

            def proj_v(t, pair):
                # V feats for heads [4*pair, 4*pair+4), token tile t (N=256)
                ps = psmix.tile([128, 256], F32, name="ps_v", tag="mix")
                for d in range(NDT):
                    nc.tensor.matmul(
                        ps[:],
                        lhsT=XT[:, d * T + t * 128: d * T + (t + 1) * 128],
                        rhs=WV[:, d * 512 + pair * 256: d * 512 + (pair + 1) * 256],
                        start=(d == 0), stop=(d == NDT - 1),
                    )
                vslice = VHAT[:, t * VROW + pair * 512: t * VROW + (pair + 1) * 512
                              ].rearrange("p (h c) -> p h c", c=128)[:, :, 64:128]
                nc.vector.tensor_add(
                    vslice,
                    ps[:].rearrange("p (h c) -> p h c", c=64),
                    BV[:, pair * 256:(pair + 1) * 256].rearrange(
                        "p (h c) -> p h c", c=64))

            def out_proj(qc2, otc2, grp):
                t4, c2 = grp // 2, grp % 2
                tt = qc2 * 4 + t4
                ps = psmix.tile([128, 512], F32, name="ps_y", tag="mix")
                for f in range(4):
                    nc.tensor.matmul(
                        ps[:],
                        lhsT=otc2[f][:, t4 * 128:(t4 + 1) * 128],
                        rhs=WO[:, f * 1024 + c2 * 512: f * 1024 + c2 * 512 + 512],
                        start=(f == 0), stop=(f == 3))
                yv = evacpool.tile([128, 512], F32, name="yv", tag="yv")
                nc.vector.tensor_add(yv[:], ps[:], BO[:, c2 * 512:(c2 + 1) * 512])
                nc.sync.dma_start(
                    y_d[tt * 128:(tt + 1) * 128, c2 * 512:(c2 + 1) * 512], yv[:])

            # ---- prologue: just enough for the exp stream to start ----
            proj_qk(4, 0)   # K feats for hp0, token chunk 0
            proj_qk(0, 0)   # Q feats for qt0, query chunk 0

            # ---- block order: interleave qc0/qc1 (then qc2/qc3) so the
            # projection fillers' deadlines spread over 128 periods instead
            # of crowding into the first 64 ----
            BLOCKS = [(0, 0), (0, 1), (1, 0), (1, 1), (0, 2), (0, 3), (1, 2), (1, 3),
                      (2, 0), (2, 1), (3, 0), (3, 1), (2, 2), (2, 3), (3, 2), (3, 3)]

            # ---- filler schedule: block index, kt -> list of thunks ----
            fillers = {}

            def add_filler(bi, kt, thunk):
                fillers.setdefault((bi, kt), []).append(thunk)

            # K feature tiles, chunk-granular just-in-time:
            #   m=4: prologue c0; c1/c2/c3 inside blk0 (first hp0 block)
            #   m=5: c0 at blk0 tail; c1/2/3 inside blk1 (first hp1 block)
            #   m=6: c0 at blk3 tail; c1/2/3 inside blk4 (first hp2 block)
            #   m=7: c0 at blk4 tail; c1/2/3 inside blk5 (first hp3 block)
            for c in range(1, 4):
                add_filler(0, 4 * c - 3, (lambda c=c: proj_qk(4, c)))
                add_filler(1, 4 * c - 3, (lambda c=c: proj_qk(5, c)))
                add_filler(4, 4 * c - 3, (lambda c=c: proj_qk(6, c)))
                add_filler(5, 4 * c - 3, (lambda c=c: proj_qk(7, c)))
            add_filler(0, 10, (lambda: proj_qk(5, 0)))
            add_filler(3, 10, (lambda: proj_qk(6, 0)))
            add_filler(4, 10, (lambda: proj_qk(7, 0)))
            # Q-tiles JIT: block bi needs Q(m=hp, c=qc); emit one block ahead
            # (kt 11, not 13-14 -- the block-tail kts already carry V+spread)
            for bi in range(1, 16):
                qc, hp = BLOCKS[bi]
                add_filler(bi - 1, 11, (lambda hp=hp, qc=qc: proj_qk(hp, qc)))
            # V projection: pair0 (heads 0..3) during blk0, pair1 during blk4
            for t in range(NKT):
                add_filler(0, t, (lambda t=t: proj_v(t, 0)))
                add_filler(4, t, (lambda t=t: proj_v(t, 1)))

            # deferred out-proj groups: qc ready after its last block's evac
            # (qc0 after blk5, qc1 after blk7, qc2 after blk13, qc3 at end).
            # Groups are spread across the NEXT block's kts as fillers so the
            # boundary never dumps a multi-us matmul burst in front of the
            # S-pairs (which would drain the 2-deep st buffer and stall ACT).
            outproj_spread = {
                6: [(0, 0), (0, 1)], 7: [(0, 2), (0, 3)], 8: [(0, 4), (0, 5)],
                9: [(0, 6), (0, 7)],
                10: [(1, 0), (1, 1)], 11: [(1, 2), (1, 3)], 12: [(1, 4), (1, 5)],
                13: [(1, 6), (1, 7), (2, 0), (2, 1)],
                14: [(2, 2), (2, 3), (2, 4), (2, 5), (2, 6), (2, 7)],
            }
            _slots = {2: (2, 8), 4: (2, 5, 8, 11), 6: (2, 4, 6, 8, 10, 12)}
            for bi, work in outproj_spread.items():
                for j, (oqc, g) in enumerate(work):
                    add_filler(bi + 1, _slots[len(work)][j],
                               (lambda oqc=oqc, g=g: out_proj(
                                   oqc, [otc_by_qc[oqc][f] for f in range(4)], g)))

            state = {}
            otc_by_qc = {}

            def emit_block_tail(bi):
                qc, hp = BLOCKS[bi]
                ops = state.pop(bi)["ops"]
                OTc = otpool.tile([128, 512], BF16, name="OTc", tag="otc")
                for sub in range(2):
                    rec = recpool.tile([64, 512], F32, name="rec", tag="rec")
                    nc.vector.reciprocal_approx_fast(rec[:], ops[sub][0:64, :])
                    nc.vector.tensor_mul(
                        OTc[sub * 64:sub * 64 + 64, :],
                        ops[sub][64:128, :], rec[:])
                otc_by_qc.setdefault(qc, {})[hp] = OTc

            def pv_pair(bi, kt, pt):
                qc, hp = BLOCKS[bi]
                ops = state[bi]["ops"]
                for sub in range(2):
                    h = 2 * hp + sub
                    nc.tensor.matmul(
                        ops[sub][:],
                        lhsT=VHAT[:, kt * VROW + h * 128: kt * VROW + (h + 1) * 128],
                        rhs=pt[:, sub * 512:(sub + 1) * 512],
                        start=(kt == 0), stop=(kt == NKT - 1))

            # ---- attention: flattened pipeline over BLOCKS x kt ----
            # Per index: S-pair, exp, prev-block tail (at kt==1), fillers,
            # then the PV pair LAGGED by one iteration so it never blocks
            # the in-order PE queue waiting on the exp or the evac.
            pts = {}
            for i in range(len(BLOCKS) * NKT):
                bi, kt = i // NKT, i % NKT
                qc, hp = BLOCKS[bi]
                qt = hp
                ktf = 4 + hp
                if kt == 0:
                    op0 = psops.tile([128, 512], F32, name="op0", tag="ops")
                    op1 = psops.tile([128, 512], F32, name="op1", tag="ops")
                    state[bi] = {"ops": (op0, op1)}

                # S^T pair (the two K=64 matmuls run concurrently via
                # base_partition-derived PE row tiling)
                st = psST.tile([128, 1024], F32, name="st", tag="st")
                for sub in range(2):
                    lo = sub * 64
                    nc.tensor.matmul(
                        st[:, sub * 512:(sub + 1) * 512],
                        lhsT=QTKT[lo:lo + 64, ktf * T + kt * 128: ktf * T + (kt + 1) * 128],
                        rhs=QTKT[lo:lo + 64, qt * T + qc * 512: qt * T + qc * 512 + 512],
                        start=True, stop=True)
                pt = ptpool.tile([128, 1024], BF16, name="pt", tag="pt")
                nc.scalar.activation(
                    pt[:], st[:], AF.Exp,
                    bias=BK[:, kt:kt + 1], scale=0.125)
                pts[(bi, kt)] = pt

                if kt == 2 and bi > 0:
                    emit_block_tail(bi - 1)
                for th in fillers.get((bi, kt), ()):
                    th()

                # PV lagged TWO iterations: its exp input is then two full
                # periods old, so the in-order PE queue never waits on ACT.
                j = i - 2
                if j >= 0:
                    bj, ktj = divmod(j, NKT)
                    pv_pair(bj, ktj, pts.pop((bj, ktj)))

            pv_pair(15, 14, pts.pop((15, 14)))
            pv_pair(15, 15, pts.pop((15, 15)))
            emit_block_tail(15)
            # qc3's out-projs drain at the end
            otc3 = [otc_by_qc[3][f] for f in range(4)]
            for grp in range(8):
                out_proj(3, otc3, grp)
    nc.compile()
    return nc


def get_program():
    if "nc" not in _cache:
        _cache["nc"] = _build_program()
    return _cache["nc"]


def shard_inputs(x, engagement, mask, qkv_w, qkv_b, out_w, out_b):
    """Build the per-core input maps (host-side layout prep only)."""
    x = np.asarray(x, dtype=np.float32)
    engagement = np.asarray(engagement, dtype=np.float32)
    maskf = np.asarray(mask).astype(np.float32)
    qkv_w = np.asarray(qkv_w, dtype=np.float32)
    qkv_b = np.asarray(qkv_b, dtype=np.float32)
    out_w = np.asarray(out_w, dtype=np.float32)
    out_b = np.asarray(out_b, dtype=np.float32)

    qkvT = qkv_w.T  # [D, 3D]
    outT = out_w.T  # [D, D]
    in_maps = []
    for cix in range(8):
        b, hg = cix // 2, cix % 2
        qcols = qkvT[:, hg * 512:(hg + 1) * 512]
        kcols = qkvT[:, 1024 + hg * 512: 1024 + (hg + 1) * 512]
        sel = np.concatenate([qcols, kcols], axis=1)  # [1024 din, 1024 feats]
        # [d, p, m, f] -> [m, p, d, f] -> [(m p), (d f)]
        wqk = sel.reshape(NDT, 128, 8, 128).transpose(2, 1, 0, 3).reshape(1024, 1024)
        bq = qkv_b[hg * 512:(hg + 1) * 512].reshape(4, 128).T
        bk = qkv_b[1024 + hg * 512: 1024 + (hg + 1) * 512].reshape(4, 128).T
        bo = np.broadcast_to(out_b, (128, 1024)) if hg == 0 else np.zeros((128, 1024), np.float32)
        small1 = np.concatenate(
            [bq, bk,
             engagement[b].reshape(NKT, 128).T,
             maskf[b].reshape(NKT, 128).T], axis=1)
        in_maps.append({
            "xt": np.ascontiguousarray(x[b].T).astype(NP_BF16),
            "wqk": np.ascontiguousarray(wqk).astype(NP_BF16),
            "wv": np.ascontiguousarray(
                qkvT[:, 2048 + hg * 512: 2048 + (hg + 1) * 512]).astype(NP_BF16),
            "small1": np.ascontiguousarray(small1),
            "bv": np.ascontiguousarray(
                np.broadcast_to(qkv_b[2048 + hg * 512: 2048 + (hg + 1) * 512], (128, 512))),
            "wo": np.ascontiguousarray(outT[hg * 512:(hg + 1) * 512, :]).astype(NP_BF16),
            "bo": np.ascontiguousarray(bo),
        })
    return in_maps


def kernel(x, engagement, mask, qkv_w, qkv_b, out_w, out_b):
    global last_results
    nc = get_program()
    in_maps = shard_inputs(x, engagement, mask, qkv_w, qkv_b, out_w, out_b)
    res = run_bass_kernel_spmd(nc, in_maps, list(range(8)))
    last_results = res
    out = np.empty((B, T, D), dtype=np.float32)
    for b in range(B):
        out[b] = res.results[2 * b]["y"] + res.results[2 * b + 1]["y"]
    return out
